# revision 1
# baseline (speedup 1.0000x reference)
"""CrossAttentionBlock Trainium2 kernel (8 NeuronCores).

Sharding: 2-way data parallel over batch x 4-way tensor parallel
(attention: 3 of 12 heads per core; MLP: 512 of 2048 tokens per core
after a ReduceScatter of the o-projection partials).

Device program (identical on all cores; per-core data via inputs):
  LN(query), LN(kv) in token-major layout (bn_stats)
  -> bf16, DMA-transpose to feature-major qn^T / kvn^T
  Q/K/V projections for this core's 3 heads (token-major out)
  2D axial RoPE on Q,K via gathered cos/sin tables (indirect DMA)
  DMA-transpose Q,K -> head-dim-major Q^T,K^T
  scores^T = K^T.T Q^T per head (fp? bf16, 2-head packed via partitions)
  probs^T = exp(scale*scores^T)  (ACT, batched from SBUF staging)
  attn^T = V_aug.T @ probs^T  (V augmented with ones column -> denominators)
  normalize via K=1 broadcast matmul of reciprocal denominators
  xo^T = wo_s^T @ attn^T  (partial over this core's heads)
  ReduceScatter(add) over the 4 cores of the batch group -> x^T token slice
  x^T += query^T + bo ; LN in feature-major layout (ones-matmul stats +
  K=1 broadcast) ; FF1 -> exact GELU -> FF2 ; residual ; out^T [768,512]
Host: stitches out[b, 512s:512(s+1), :] = out_t.T
"""

import numpy as np
import ml_dtypes

import concourse.bass as bass
import concourse.tile as tile
import concourse.mybir as mybir

BF = mybir.dt.bfloat16
F32 = mybir.dt.float32
I32 = mybir.dt.int32
AF = mybir.ActivationFunctionType
ALU = mybir.AluOpType

DIM = 768
H = 12
DH = 64
MLP_H = 3072
B = 2
P = 2048          # both PQ and PKV
N_CORES = 8
G = 4             # cores per batch group
HPC = H // G      # heads per core = 3
TPC = P // G      # tokens per core for MLP = 512
ROPE_THETA = 100.0
LN_EPS = 1e-5
GROUPS = [[0, 1, 2, 3], [4, 5, 6, 7]]

NT = P // 128     # 16 token tiles
KT = DIM // 128   # 6 feature tiles
QC = 512          # q chunk in attention
NQC = P // QC     # 4


def _split_multi_waits(nc, max_waits: int = 1):
    """Walrus codegen in this container accepts at most one sync wait per
    instruction; Tile's post-scheduler drain/barrier can carry more. Move
    the excess onto same-engine nops inserted just before."""
    for bb in nc.main_func.blocks:
        i = 0
        insts = bb.instructions
        while i < len(insts):
            ins = insts[i]
            si = ins.sync_info
            if si is not None and si.on_wait and len(si.on_wait) > max_waits:
                waits = list(si.on_wait)
                keep = waits[-max_waits:]
                extra = waits[:-max_waits]
                nops = []
                for w in extra:
                    nop = mybir.InstNoOp(
                        name=f"I-waitfix-{nc.next_id()}", engine=ins.engine
                    )
                    nop.sync_info = mybir.SyncInfo(on_wait=[w], on_update=[])
                    nops.append(nop)
                ins.sync_info = mybir.SyncInfo(
                    on_wait=keep, on_update=list(si.on_update or [])
                )
                for j, nop in enumerate(nops):
                    insts.insert(i + j, nop)
                i += len(nops)
            i += 1


def build_nc():
    nc = bass.Bass("TRN2", target_bir_lowering=False, debug=False,
                   num_devices=N_CORES)

    # ---------------- inputs ----------------
    query = nc.dram_tensor("query", [P, DIM], F32, kind="ExternalInput")
    kv = nc.dram_tensor("kv", [P, DIM], F32, kind="ExternalInput")
    q_res_t = nc.dram_tensor("q_res_t", [DIM, TPC], F32, kind="ExternalInput")
    posq = nc.dram_tensor("posq", [128, NT, 2], I32, kind="ExternalInput")
    poskv = nc.dram_tensor("poskv", [128, NT, 2], I32, kind="ExternalInput")
    trig = nc.dram_tensor("trig", [64, 32], F32, kind="ExternalInput")
    lnq_w = nc.dram_tensor("lnq_w", [DIM], F32, kind="ExternalInput")
    lnq_b = nc.dram_tensor("lnq_b", [DIM], F32, kind="ExternalInput")
    lnkv_w = nc.dram_tensor("lnkv_w", [DIM], F32, kind="ExternalInput")
    lnkv_b = nc.dram_tensor("lnkv_b", [DIM], F32, kind="ExternalInput")
    wq_s = nc.dram_tensor("wq_s", [DIM, HPC * DH], BF, kind="ExternalInput")
    wkv_s = nc.dram_tensor("wkv_s", [DIM, 2 * HPC * DH], BF, kind="ExternalInput")
    bqkv_s = nc.dram_tensor("bqkv_s", [3 * HPC * DH], F32, kind="ExternalInput")
    wo_s = nc.dram_tensor("wo_s", [HPC * DH, DIM], BF, kind="ExternalInput")
    bo = nc.dram_tensor("bo", [DIM], F32, kind="ExternalInput")
    lnm_w = nc.dram_tensor("lnm_w", [DIM], F32, kind="ExternalInput")
    lnm_b = nc.dram_tensor("lnm_b", [DIM], F32, kind="ExternalInput")
    w1 = nc.dram_tensor("w1", [DIM, MLP_H], BF, kind="ExternalInput")
    b1 = nc.dram_tensor("b1", [MLP_H], F32, kind="ExternalInput")
    w2 = nc.dram_tensor("w2", [MLP_H, DIM], BF, kind="ExternalInput")
    b2 = nc.dram_tensor("b2", [DIM], F32, kind="ExternalInput")
    out_t = nc.dram_tensor("out_t", [DIM, TPC], F32, kind="ExternalOutput")

    def bcast_ap(t, n_part, free):
        return bass.AP(tensor=t.ap().tensor, offset=0,
                       ap=[[0, n_part], [1, free]])

    IB = 2  # kv tiles per exp batch

    with tile.TileContext(nc) as tc:
        with (
            tc.tile_pool(name="consts", bufs=1) as consts,
            tc.tile_pool(name="mlpw", bufs=1) as mlpw,
            tc.tile_pool(name="work", bufs=3) as work,
            tc.tile_pool(name="dram", bufs=1, space="DRAM") as dram,
        ):
            # ---------------- constants ----------------
            ones_bf = consts.tile([128, 128], BF)
            nc.vector.memset(ones_bf[:], 1.0)
            eps_sb = consts.tile([128, 1], F32)
            nc.vector.memset(eps_sb[:], LN_EPS)
            bo_sb = consts.tile([128, KT], F32)
            nc.sync.dma_start(bo_sb[:], bo.rearrange("(m p) -> p m", p=128))
            lnmw_sb = consts.tile([128, KT], F32)
            nc.sync.dma_start(lnmw_sb[:], lnm_w.rearrange("(m p) -> p m", p=128))
            lnmb_sb = consts.tile([128, KT], F32)
            nc.sync.dma_start(lnmb_sb[:], lnm_b.rearrange("(m p) -> p m", p=128))
            b1_sb = consts.tile([128, MLP_H // 128], F32)
            nc.sync.dma_start(b1_sb[:], b1.rearrange("(m p) -> p m", p=128))
            b2_sb = consts.tile([128, KT], F32)
            nc.sync.dma_start(b2_sb[:], b2.rearrange("(m p) -> p m", p=128))
            posq_sb = consts.tile([128, NT, 2], I32)
            nc.sync.dma_start(posq_sb[:], posq[:])
            poskv_sb = consts.tile([128, NT, 2], I32)
            nc.sync.dma_start(poskv_sb[:], poskv[:])

            # MLP weights (slab lives whole kernel; DMA overlaps earlier work)
            w1_sb = mlpw.tile([128, KT, MLP_H], BF)
            nc.sync.dma_start(w1_sb[:], w1.rearrange("(k p) n -> p k n", p=128))
            w2_sb = mlpw.tile([128, MLP_H // 128, DIM], BF)
            nc.sync.dma_start(w2_sb[:], w2.rearrange("(k p) n -> p k n", p=128))

            qn_dram = dram.tile([P, DIM], BF)
            kvn_dram = dram.tile([P, DIM], BF)
            qrot_dram = dram.tile([P, 256], BF)
            krot_dram = dram.tile([P, 256], BF)
            cc_in = dram.tile([G, DIM, TPC], F32)
            cc_out = dram.tile([DIM, TPC], F32)

            with tc.tile_pool(name="attnP", bufs=1) as attnP:
                # attention-lifetime tiles
                wq_sb = attnP.tile([128, KT, HPC * DH], BF)
                nc.sync.dma_start(wq_sb[:],
                                  wq_s.rearrange("(k p) n -> p k n", p=128))
                wkv_sb = attnP.tile([128, KT, 2 * HPC * DH], BF)
                nc.sync.dma_start(wkv_sb[:],
                                  wkv_s.rearrange("(k p) n -> p k n", p=128))
                wo01_sb = attnP.tile([128, DIM], BF)
                nc.sync.dma_start(wo01_sb[:], wo_s[0:128, :])
                wo2_sb = attnP.tile([64, DIM], BF)
                nc.sync.dma_start(wo2_sb[:], wo_s[128:192, :])
                bqkv_rep = attnP.tile([128, 3 * HPC * DH], F32)
                nc.sync.dma_start(bqkv_rep[:],
                                  bcast_ap(bqkv_s, 128, 3 * HPC * DH))
                v_aug = attnP.tile([128, NT, HPC, DH + 1], BF)
                nc.vector.memset(v_aug[:, :, :, DH:DH + 1], 1.0)
                qT01 = attnP.tile([128, P], BF)
                qT2 = attnP.tile([128, P], BF)
                kT01 = attnP.tile([128, P], BF)
                kT2 = attnP.tile([128, P], BF)
                at01 = attnP.tile([128, P], BF)
                at2 = attnP.tile([64, P], BF)
                h1_stage = attnP.tile([64, P], BF)

                # ------------ phase 1: LN + transposes + proj + RoPE ------------
                with (
                    tc.tile_pool(name="earlyP", bufs=1) as earlyP,
                    tc.tile_pool(name="lnwk", bufs=2) as lnwk,
                    tc.tile_pool(name="psProj", bufs=2, space="PSUM") as psProj,
                ):
                    def layer_norm_side(src, w_dram, b_dram, dst):
                        wrep = lnwk.tile([128, DIM], BF, tag="wrep")
                        nc.gpsimd.dma_start(wrep[:], bcast_ap(w_dram, 128, DIM))
                        brep = lnwk.tile([128, DIM], BF, tag="brep")
                        nc.gpsimd.dma_start(brep[:], bcast_ap(b_dram, 128, DIM))
                        for t in range(NT):
                            xt = lnwk.tile([128, DIM], F32, tag="lnx")
                            nc.sync.dma_start(
                                xt[:], src[t * 128:(t + 1) * 128, :])
                            st = lnwk.tile([128, 3, nc.vector.BN_STATS_DIM],
                                           F32, tag="bnst")
                            xg = xt[:].rearrange("p (g d) -> p g d", g=3)
                            for g in range(3):
                                nc.vector.bn_stats(st[:, g, :], xg[:, g, :])
                            mv = lnwk.tile([128, nc.vector.BN_AGGR_DIM], F32,
                                           tag="bnmv")
                            nc.vector.bn_aggr(mv[:], st[:])
                            rs = lnwk.tile([128, 1], F32, tag="lnrs")
                            nc.scalar.activation(rs[:], mv[:, 1:2], AF.Sqrt,
                                                 bias=eps_sb[:], scale=1.0)
                            nc.vector.reciprocal(rs[:], rs[:])
                            nsc = lnwk.tile([128, 1], F32, tag="lnns")
                            nc.vector.tensor_mul(nsc[:], mv[:, 0:1], rs[:])
                            nc.vector.tensor_scalar_mul(nsc[:], nsc[:], -1.0)
                            # x = (x - m) * r  (ACT free affine), then *w + b
                            nc.scalar.activation(xt[:], xt[:], AF.Identity,
                                                 bias=nsc[:], scale=rs[:])
                            nc.vector.tensor_mul(xt[:], xt[:], wrep[:])
                            xbf = lnwk.tile([128, DIM], BF, tag="lnbf")
                            nc.vector.tensor_add(xbf[:], xt[:], brep[:])
                            nc.sync.dma_start(
                                dst[t * 128:(t + 1) * 128, :], xbf[:])

                    def rope_apply(nc, src_f32, gt, dst_bf):
                        s5 = src_f32[:].rearrange(
                            "p (h a j two) -> p h a j two",
                            h=HPC, a=2, j=16, two=2)
                        d5 = dst_bf[:].rearrange(
                            "p (h a j two) -> p h a j two",
                            h=HPC, a=2, j=16, two=2)
                        cos3 = gt[:, None, :, 0:16].to_broadcast(
                            (128, HPC, 2, 16))
                        sin3 = gt[:, None, :, 16:32].to_broadcast(
                            (128, HPC, 2, 16))
                        xe = s5[:, :, :, :, 0]
                        xo = s5[:, :, :, :, 1]
                        ta = work.tile([128, HPC, 2, 16], F32, tag="rta")
                        tb = work.tile([128, HPC, 2, 16], F32, tag="rtb")
                        nc.vector.tensor_mul(ta[:], xe, cos3)
                        nc.vector.tensor_mul(tb[:], xo, sin3)
                        nc.vector.tensor_tensor(d5[:, :, :, :, 0], ta[:],
                                                tb[:], ALU.subtract)
                        nc.vector.tensor_mul(ta[:], xe, sin3)
                        nc.vector.tensor_mul(tb[:], xo, cos3)
                        nc.vector.tensor_tensor(d5[:, :, :, :, 1], ta[:],
                                                tb[:], ALU.add)

                    def proj_side(nT, is_q):
                        for t in range(NT):
                            gt = work.tile([128, 2, 32], F32,
                                           tag="gq" if is_q else "gk")
                            pos_sb = posq_sb if is_q else poskv_sb
                            for a in range(2):
                                nc.gpsimd.indirect_dma_start(
                                    out=gt[:, a, :], out_offset=None,
                                    in_=trig[:],
                                    in_offset=bass.IndirectOffsetOnAxis(
                                        ap=pos_sb[:, t, a:a + 1], axis=0))
                            if is_q:
                                q_ps = psProj.tile([128, HPC * DH], F32,
                                                   tag="qps")
                                for k in range(KT):
                                    nc.tensor.matmul(
                                        q_ps[:],
                                        nT[:, k, t * 128:(t + 1) * 128],
                                        wq_sb[:, k, :], start=(k == 0),
                                        stop=(k == KT - 1))
                                qb = work.tile([128, HPC * DH], F32, tag="qb")
                                nc.vector.tensor_add(
                                    qb[:], q_ps[:], bqkv_rep[:, 0:HPC * DH])
                                q_rot = work.tile([128, HPC * DH], BF,
                                                  tag="qrot")
                                rope_apply(nc, qb, gt, q_rot)
                                nc.sync.dma_start(
                                    qrot_dram[t * 128:(t + 1) * 128,
                                              0:HPC * DH], q_rot[:])
                            else:
                                kv_ps = psProj.tile([128, 2 * HPC * DH], F32,
                                                    tag="kvps")
                                for k in range(KT):
                                    nc.tensor.matmul(
                                        kv_ps[:],
                                        nT[:, k, t * 128:(t + 1) * 128],
                                        wkv_sb[:, k, :], start=(k == 0),
                                        stop=(k == KT - 1))
                                kb = work.tile([128, HPC * DH], F32, tag="kb")
                                nc.vector.tensor_add(
                                    kb[:], kv_ps[:, 0:HPC * DH],
                                    bqkv_rep[:, HPC * DH:2 * HPC * DH])
                                k_rot = work.tile([128, HPC * DH], BF,
                                                  tag="krot")
                                rope_apply(nc, kb, gt, k_rot)
                                nc.sync.dma_start(
                                    krot_dram[t * 128:(t + 1) * 128,
                                              0:HPC * DH], k_rot[:])
                                nc.vector.tensor_tensor(
                                    v_aug[:, t, :, 0:DH],
                                    kv_ps[:, HPC * DH:2 * HPC * DH].rearrange(
                                        "p (h d) -> p h d", h=HPC),
                                    bqkv_rep[:,
                                             2 * HPC * DH:3 * HPC * DH
                                             ].rearrange(
                                        "p (h d) -> p h d", h=HPC), ALU.add)

                    # q side, then kv side (shared nT slab, tag reuse)
                    layer_norm_side(query, lnq_w, lnq_b, qn_dram)
                    nT_q = earlyP.tile([128, KT, P], BF, tag="nT", name="nTq")
                    for m in range(KT):
                        nc.sync.dma_start_transpose(
                            nT_q[:, m, :], qn_dram[:, m * 128:(m + 1) * 128])
                    proj_side(nT_q, True)

                    layer_norm_side(kv, lnkv_w, lnkv_b, kvn_dram)
                    nT_kv = earlyP.tile([128, KT, P], BF, tag="nT", name="nTkv")
                    for m in range(KT):
                        nc.sync.dma_start_transpose(
                            nT_kv[:, m, :], kvn_dram[:, m * 128:(m + 1) * 128])
                    proj_side(nT_kv, False)

                # Q,K -> head-dim-major
                nc.sync.dma_start_transpose(qT01[:], qrot_dram[:, 0:128])
                nc.sync.dma_start_transpose(qT2[:], qrot_dram[:, 128:256])
                nc.sync.dma_start_transpose(kT01[:], krot_dram[:, 0:128])
                nc.sync.dma_start_transpose(kT2[:], krot_dram[:, 128:256])

                # ------------ phase 3: attention ------------
                with (
                    tc.tile_pool(name="stageP", bufs=2) as stageP,
                    tc.tile_pool(name="probsP", bufs=2) as probsP,
                    tc.tile_pool(name="psS", bufs=2, space="PSUM") as psS,
                    tc.tile_pool(name="psAttn", bufs=1, space="PSUM") as psAttn,
                    tc.tile_pool(name="psRep", bufs=1, space="PSUM") as psRep,
                ):
                    for h in range(HPC):
                        if h == 0:
                            qTh, kTh = qT01[0:64, :], kT01[0:64, :]
                        elif h == 1:
                            qTh, kTh = qT01[64:128, :], kT01[64:128, :]
                        else:
                            qTh, kTh = qT2[0:64, :], kT2[0:64, :]

                        attn_ps = [psAttn.tile([65, QC], F32, tag=f"attn{c}",
                                               name=f"attn_{h}_{c}")
                                   for c in range(NQC)]
                        for ib in range(NT // IB):
                            st_stage = stageP.tile([128, IB, NQC, QC], F32,
                                                   tag="sstage")
                            for ii in range(IB):
                                i = ib * IB + ii
                                for c in range(NQC):
                                    s_ps = psS.tile([128, QC], F32, tag="sps")
                                    nc.tensor.matmul(
                                        s_ps[:], kTh[:, i * 128:(i + 1) * 128],
                                        qTh[:, c * QC:(c + 1) * QC],
                                        start=True, stop=True)
                                    nc.vector.tensor_copy(
                                        st_stage[:, ii, c, :], s_ps[:])
                            prb = probsP.tile([128, IB, NQC, QC], BF,
                                              tag="prb")
                            nc.scalar.activation(
                                prb[:].rearrange("p a b n -> p (a b n)"),
                                st_stage[:].rearrange("p a b n -> p (a b n)"),
                                AF.Exp, bias=0.0, scale=DH ** (-0.5))
                            for ii in range(IB):
                                i = ib * IB + ii
                                for c in range(NQC):
                                    nc.tensor.matmul(
                                        attn_ps[c][:], v_aug[:, i, h, :],
                                        prb[:, ii, c, :], start=(i == 0),
                                        stop=(i == NT - 1))

                        # normalize
                        if h == 0:
                            dst = at01[0:64, :]
                        elif h == 1:
                            dst = h1_stage[:]
                        else:
                            dst = at2[:]
                        for c in range(NQC):
                            rcp = work.tile([128, QC], BF, tag="rcp")
                            with nc.allow_low_precision(
                                    reason="softmax denom recip in bf16 "
                                           "matches bf16 matmul precision"):
                                nc.vector.reciprocal(
                                    rcp[64:65, :], attn_ps[c][64:65, :])
                            rep_ps = psRep.tile([64, QC], F32, tag="rep")
                            nc.tensor.matmul(rep_ps[:], ones_bf[64:65, 0:64],
                                             rcp[64:65, :],
                                             start=True, stop=True)
                            rep_sb = work.tile([64, QC], BF, tag="repsb")
                            nc.vector.tensor_copy(rep_sb[:], rep_ps[:])
                            nc.vector.tensor_mul(
                                dst[:, c * QC:(c + 1) * QC],
                                attn_ps[c][0:64, :], rep_sb[:])
                        if h == 1:
                            nc.sync.dma_start(at01[64:128, :], h1_stage[:])

                # ------------ phase 4: o-proj + ReduceScatter ------------
                with tc.tile_pool(name="psO", bufs=3, space="PSUM") as psO:
                    for m in range(KT):
                        for c in range(NQC):
                            xo_ps = psO.tile([128, QC], F32, tag="xops")
                            nc.tensor.matmul(
                                xo_ps[:], wo01_sb[:, m * 128:(m + 1) * 128],
                                at01[:, c * QC:(c + 1) * QC],
                                start=True, stop=False)
                            nc.tensor.matmul(
                                xo_ps[:], wo2_sb[:, m * 128:(m + 1) * 128],
                                at2[:, c * QC:(c + 1) * QC],
                                start=False, stop=True)
                            xo_sb = work.tile([128, QC], F32, tag="xosb")
                            nc.vector.tensor_copy(xo_sb[:], xo_ps[:])
                            nc.sync.dma_start(
                                cc_in[c, m * 128:(m + 1) * 128, :], xo_sb[:])

            nc.gpsimd.collective_compute(
                "ReduceScatter", ALU.add, replica_groups=GROUPS,
                ins=[cc_in[:].opt()], outs=[cc_out[:].opt()])

            # ------------ phase 5: residual + MLP ------------
            with tc.tile_pool(name="mlpP", bufs=1) as mlpP:
                x_sb = mlpP.tile([128, KT, TPC], F32)
                nc.sync.dma_start(
                    x_sb[:], cc_out[:].rearrange("(m p) n -> p m n", p=128))
                with tc.tile_pool(name="mlptmp", bufs=1) as mlptmp:
                    qres_sb = mlptmp.tile([128, KT, TPC], F32)
                    nc.sync.dma_start(
                        qres_sb[:],
                        q_res_t[:].rearrange("(m p) n -> p m n", p=128))
                    for m in range(KT):
                        nc.vector.tensor_add(x_sb[:, m, :], x_sb[:, m, :],
                                             qres_sb[:, m, :])
                        nc.vector.tensor_scalar(x_sb[:, m, :], x_sb[:, m, :],
                                                bo_sb[:, m:m + 1], None,
                                                ALU.add, ALU.bypass)

                    # LN (feature-major)
                    xb = mlptmp.tile([128, KT, TPC], BF)
                    sqb = mlptmp.tile([128, KT, TPC], BF)
                    for m in range(KT):
                        nc.vector.tensor_copy(xb[:, m, :], x_sb[:, m, :])
                        nc.vector.tensor_mul(sqb[:, m, :], xb[:, m, :],
                                             xb[:, m, :])
                    xn_sb = mlpP.tile([128, KT, TPC], BF)
                    with tc.tile_pool(name="psStat", bufs=1,
                                      space="PSUM") as psStat:
                        mean_ps = psStat.tile([1, TPC], F32, tag="meanps")
                        sq_ps = psStat.tile([1, TPC], F32, tag="sqps")
                        for m in range(KT):
                            nc.tensor.matmul(mean_ps[:], ones_bf[:, 0:1],
                                             xb[:, m, :], start=(m == 0),
                                             stop=(m == KT - 1))
                        for m in range(KT):
                            nc.tensor.matmul(sq_ps[:], ones_bf[:, 0:1],
                                             sqb[:, m, :], start=(m == 0),
                                             stop=(m == KT - 1))
                        mrow_bf = mlptmp.tile([1, TPC], BF)
                        rrow_bf = mlptmp.tile([1, TPC], BF)
                        mrow = mlptmp.tile([1, TPC], F32)
                        vrow = mlptmp.tile([1, TPC], F32)
                        nc.vector.tensor_scalar_mul(mrow[:], mean_ps[:],
                                                    1.0 / DIM)
                        nc.vector.tensor_scalar_mul(vrow[:], sq_ps[:],
                                                    1.0 / DIM)
                        msq = mlptmp.tile([1, TPC], F32)
                        nc.vector.tensor_mul(msq[:], mrow[:], mrow[:])
                        nc.vector.tensor_tensor(vrow[:], vrow[:], msq[:],
                                                ALU.subtract)
                        nc.scalar.activation(vrow[:], vrow[:], AF.Sqrt,
                                             bias=eps_sb[0:1, :], scale=1.0)
                        nc.vector.reciprocal(vrow[:], vrow[:])
                        nc.vector.tensor_copy(rrow_bf[:], vrow[:])
                        nc.vector.tensor_copy(mrow_bf[:], mrow[:])
                        with tc.tile_pool(name="psReps", bufs=1,
                                          space="PSUM") as psReps:
                            mrep_ps = psReps.tile([128, TPC], F32, tag="mrep")
                            nc.tensor.matmul(mrep_ps[:], ones_bf[0:1, :],
                                             mrow_bf[:], start=True, stop=True)
                            rrep_ps = psReps.tile([128, TPC], F32, tag="rrep")
                            nc.tensor.matmul(rrep_ps[:], ones_bf[0:1, :],
                                             rrow_bf[:], start=True, stop=True)
                            for m in range(KT):
                                t1 = work.tile([128, TPC], F32, tag="mlnt1")
                                nc.vector.tensor_tensor(
                                    t1[:], x_sb[:, m, :], mrep_ps[:],
                                    ALU.subtract)
                                nc.vector.tensor_mul(t1[:], t1[:], rrep_ps[:])
                                nc.vector.tensor_scalar(
                                    xn_sb[:, m, :], t1[:],
                                    lnmw_sb[:, m:m + 1], lnmb_sb[:, m:m + 1],
                                    ALU.mult, ALU.add)

                # FF1 + GELU
                h_sb = mlpP.tile([128, MLP_H // 128, TPC], BF)
                with tc.tile_pool(name="psF1", bufs=1, space="PSUM") as psF1:
                    for jp in range(MLP_H // 256):
                        f1 = psF1.tile([128, 2, TPC], F32, tag="f1")
                        for jj in range(2):
                            j = jp * 2 + jj
                            for k in range(KT):
                                nc.tensor.matmul(
                                    f1[:, jj, :],
                                    w1_sb[:, k, j * 128:(j + 1) * 128],
                                    xn_sb[:, k, :], start=(k == 0),
                                    stop=(k == KT - 1))
                            nc.vector.tensor_scalar(
                                f1[:, jj, :], f1[:, jj, :], b1_sb[:, j:j + 1],
                                None, ALU.add, ALU.bypass)
                        nc.scalar.activation(
                            h_sb[:, jp * 2:jp * 2 + 2, :].rearrange(
                                "p a n -> p (a n)"),
                            f1[:].rearrange("p a n -> p (a n)"), AF.Gelu)

                    # FF2 + residual + out
                    with tc.tile_pool(name="psF2", bufs=1,
                                      space="PSUM") as psF2:
                        f2 = [psF2.tile([128, TPC], F32, tag=f"f2_{m}",
                                        name=f"f2t_{m}")
                              for m in range(KT)]
                        for j in range(MLP_H // 128):
                            for m in range(KT):
                                nc.tensor.matmul(
                                    f2[m][:],
                                    w2_sb[:, j, m * 128:(m + 1) * 128],
                                    h_sb[:, j, :], start=(j == 0),
                                    stop=(j == MLP_H // 128 - 1))
                        for m in range(KT):
                            fo = work.tile([128, TPC], F32, tag="fo")
                            nc.vector.tensor_tensor(fo[:], f2[m][:],
                                                    x_sb[:, m, :], ALU.add)
                            nc.vector.tensor_scalar(fo[:], fo[:],
                                                    b2_sb[:, m:m + 1], None,
                                                    ALU.add, ALU.bypass)
                            nc.sync.dma_start(
                                out_t[m * 128:(m + 1) * 128, :], fo[:])

    _split_multi_waits(nc)
    return nc


_NC_CACHE = None


def _get_nc():
    global _NC_CACHE
    if _NC_CACHE is None:
        _NC_CACHE = build_nc()
    return _NC_CACHE


def _make_trig():
    j = np.arange(16)
    f = 1.0 / (ROPE_THETA ** (2.0 * j / 32.0))
    v = np.arange(64)
    ang = v[:, None] * f[None, :]
    return np.concatenate([np.cos(ang), np.sin(ang)], axis=1).astype(np.float32)


def kernel(**inputs):
    from concourse.bass_utils import run_bass_kernel_spmd

    np32 = lambda x: np.asarray(x, dtype=np.float32)
    npbf = lambda x: np.asarray(np.asarray(x, dtype=np.float32),
                                dtype=ml_dtypes.bfloat16)
    query = np32(inputs["query"])
    kv = np32(inputs["kv"])
    pos_q = np.asarray(inputs["pos_q"]).astype(np.int32)
    pos_kv = np.asarray(inputs["pos_kv"]).astype(np.int32)
    wq, wk, wv = npbf(inputs["wq"]), npbf(inputs["wk"]), npbf(inputs["wv"])
    wo = npbf(inputs["wo"])
    w1, w2 = npbf(inputs["w1"]), npbf(inputs["w2"])
    trig = _make_trig()

    in_maps = []
    for c in range(N_CORES):
        b, s = c // G, c % G
        hs = slice(HPC * DH * s, HPC * DH * (s + 1))
        ts = slice(TPC * s, TPC * (s + 1))
        in_maps.append({
            "query": query[b],
            "kv": kv[b],
            "q_res_t": np.ascontiguousarray(query[b, ts, :].T),
            "posq": np.ascontiguousarray(
                pos_q[b].reshape(NT, 128, 2).transpose(1, 0, 2)),
            "poskv": np.ascontiguousarray(
                pos_kv[b].reshape(NT, 128, 2).transpose(1, 0, 2)),
            "trig": trig,
            "lnq_w": np32(inputs["ln_q_w"]), "lnq_b": np32(inputs["ln_q_b"]),
            "lnkv_w": np32(inputs["ln_kv_w"]), "lnkv_b": np32(inputs["ln_kv_b"]),
            "wq_s": np.ascontiguousarray(wq[:, hs]),
            "wkv_s": np.ascontiguousarray(
                np.concatenate([wk[:, hs], wv[:, hs]], axis=1)),
            "bqkv_s": np.concatenate(
                [np32(inputs["bq"])[hs], np32(inputs["bk"])[hs],
                 np32(inputs["bv"])[hs]]),
            "wo_s": np.ascontiguousarray(wo[hs, :]),
            "bo": np32(inputs["bo"]),
            "lnm_w": np32(inputs["ln_mlp_w"]), "lnm_b": np32(inputs["ln_mlp_b"]),
            "w1": w1, "b1": np32(inputs["b1"]),
            "w2": w2, "b2": np32(inputs["b2"]),
        })

    nc = _get_nc()
    res = run_bass_kernel_spmd(nc, in_maps, core_ids=list(range(N_CORES)))

    out = np.empty((B, P, DIM), np.float32)
    for c in range(N_CORES):
        b, s = c // G, c % G
        out[b, TPC * s:TPC * (s + 1), :] = res.results[c]["out_t"].T
    return out

